# revision 22
# baseline (speedup 1.0000x reference)
"""CoAtten2 Trainium2 kernel: 8-way tensor-parallel over one TRN2 chip.

Reference computation (C=1024, H=W=64, HW=4096):
    q   = (Wq @ Xm + bq)  viewed [1024, 2048] then transposed
    kf  = (Wk1 @ Xf + bk1) viewed [1024, 2048]
    kl  = (Wk2 @ Xl + bk2) viewed [1024, 2048]
    att = softmax(kf @ q) + softmax(kl @ q)          # [1024, 1024]
    out = gamma * (att @ (Wv @ Xm + bv)) + (Xf + Xl)/2

Decomposition (per core d of 8; group t = d//4, a = d%4):
  - Channel indices are permuted (I' = 512t + o <-> i = 2o + t) so the
    torch-style reshape becomes contiguous; the permutation is folded into the
    host-side Wv/bv prep and the output DMA access pattern; gamma into Wv/bv.
  - logits_PERM splits into parity quadrants Q(t, t') whose kf operand needs
    spatial columns [2048t, 2048t+2048) and whose q operand needs spatial
    columns [2048t', ...). Core d owns spatial slice S_d = [512d, 512(d+1));
    it computes the partial contraction over S_d of Q(t=d//4, t'=0 and 1) for
    both attention branches, using ONLY local projections:
      CkfT_d / CklT_d from its own x slices, and CqT over spatial blocks
      (d%4) and (4 + d%4) from two host-provided Xm slices (xq0/xq1).
  - One 4-way ReduceScatter (groups [0-3], [4-7]) then sums the four partials
    AND deals each core its 128-row attention block at a fixed local address.
  - softmax is a free-dim reduction; the summed attention is AllGathered in
    bf16; each core computes its output column slice att @ V_d + residual.
  - Collectives: one ReduceScatter (4 MB) + one small AllGather. All compute
    is local; the SPMD program is identical on every core (per-core identity
    enters only through input data).
"""

import sys

sys.path.insert(0, "/opt/trn_rl_repo")

import numpy as np

import concourse.bacc as bacc
import concourse.mybir as mybir
from concourse import tile
from concourse.tile import add_dep_helper
from concourse.bass_utils import run_bass_kernel_spmd

F32 = mybir.dt.float32
F32R = mybir.dt.float32r
BF16 = mybir.dt.bfloat16

C = 1024
HW = 4096
S = 512          # spatial columns per core
CH = 512         # C // 2 (projection output channels)
NCORES = 8

_CACHE: dict = {}


def _build():
    nc = bacc.Bacc("TRN2", target_bir_lowering=False, debug=False, num_devices=NCORES)

    # per-core external inputs
    xm = nc.declare_dram_parameter("xm", [C, S], F32, isOutput=False)
    xf = nc.declare_dram_parameter("xf", [C, S], F32, isOutput=False)
    xl = nc.declare_dram_parameter("xl", [C, S], F32, isOutput=False)
    xq0 = nc.declare_dram_parameter("xq0", [C, S], F32, isOutput=False)  # Xm block d%4
    xq1 = nc.declare_dram_parameter("xq1", [C, S], F32, isOutput=False)  # Xm block 4+d%4
    wq = nc.declare_dram_parameter("wq", [C, CH], F32, isOutput=False)   # Wq.T
    wk1 = nc.declare_dram_parameter("wk1", [C, CH], F32, isOutput=False)
    wk2 = nc.declare_dram_parameter("wk2", [C, CH], F32, isOutput=False)
    wv = nc.declare_dram_parameter("wv", [C, C], F32, isOutput=False)    # (g*Wv)[permJ].T
    bqr = nc.declare_dram_parameter("bqr", [128, CH], F32, isOutput=False)
    bk1r = nc.declare_dram_parameter("bk1r", [128, CH], F32, isOutput=False)
    bk2r = nc.declare_dram_parameter("bk2r", [128, CH], F32, isOutput=False)
    bvp = nc.declare_dram_parameter("bvp", [128, 8], F32, isOutput=False)
    xfp = nc.declare_dram_parameter("xfp", [C, S], F32, isOutput=False)  # perm rows
    xlp = nc.declare_dram_parameter("xlp", [C, S], F32, isOutput=False)
    out_ext = nc.declare_dram_parameter("out", [C, S], F32, isOutput=True)

    # internal DRAM
    rs_in_f = nc.dram_tensor("rs_in_f", [CH, C], F32)   # quadrant partials
    rs_in_l = nc.dram_tensor("rs_in_l", [CH, C], F32)
    rs_out_f = nc.dram_tensor("rs_out_f", [128, C], F32)
    rs_out_l = nc.dram_tensor("rs_out_l", [128, C], F32)
    att_in = nc.dram_tensor("att_in", [128, C], BF16)
    att_out = nc.dram_tensor("att_out", [C, C], BF16, addr_space="Shared")

    groups8 = [list(range(NCORES))]
    groups4 = [[0, 1, 2, 3], [4, 5, 6, 7]]

    with tile.TileContext(nc) as tc:
        with (
            tc.tile_pool(name="pw", bufs=1) as pw,
            tc.tile_pool(name="psg", bufs=2) as psg,
            tc.tile_pool(name="psc", bufs=2) as psc,
            tc.tile_pool(name="pps", bufs=2, space="PSUM") as pps,
            tc.tile_pool(name="plog", bufs=1, space="PSUM") as plog,
        ):
            # ---- input loads -------------------------------------------------
            def load8(dram, width, tag, dt=F32R):
                ts = []
                for c in range(8):
                    t = pw.tile([128, width], dt, tag=f"{tag}{c}")
                    nc.sync.dma_start(t[:], dram[128 * c:128 * (c + 1), :].bitcast(dt))
                    ts.append(t)
                return ts

            # ---- local transposed projections -------------------------------
            # proj(X, WT, b)[s, o] = sum_c X[c, s] WT[c, o] + b[o]  -> [512, 512]
            # result stays in SBUF as 4 [128, 512] f32r tiles (s on partitions).
            def proj(x_tiles, w_tiles, bias_t, otag):
                outs = []
                for ssub in range(4):
                    ps = pps.tile([128, CH], F32, tag="mm")
                    for c in range(8):
                        nc.tensor.matmul(
                            ps[:],
                            x_tiles[c][:, 128 * ssub:128 * (ssub + 1)],
                            w_tiles[c][:],
                            start=(c == 0),
                            stop=(c == 7),
                        )
                    o = pw.tile([128, CH], F32R, tag=f"{otag}{ssub}")
                    nc.vector.tensor_add(o[:], ps[:], bias_t[:])
                    outs.append(o)
                return outs

            def partials(ck, cq, rin, ptag):
                # For o-tile m: partial[128 o, 512 t'-block] over local s
                for m in range(4):
                    psl = plog.tile([128, C], F32, tag=ptag)
                    for tp in range(2):
                        for k in range(4):
                            nc.tensor.matmul(
                                psl[:, CH * tp:CH * (tp + 1)],
                                ck[k][:, 128 * m:128 * (m + 1)],
                                cq[tp][k][:],
                                start=(k == 0),
                                stop=(k == 3),
                            )
                    stg = psg.tile([128, C], F32, tag="stg")
                    nc.vector.tensor_copy(stg[:], psl[:])
                    nc.sync.dma_start(rin[128 * m:128 * (m + 1), :], stg[:])

            # f-branch chain first: its partials run while the l-branch's
            # inputs load and project
            xf_t = load8(xf, S, "xf")
            wk1_t = load8(wk1, CH, "wk1")
            bias_t = pw.tile([128, CH], F32, tag="bk1")
            nc.sync.dma_start(bias_t[:], bk1r[:, :])
            ckf = proj(xf_t, wk1_t, bias_t, "ckf")

            wq_t = load8(wq, CH, "wq")
            bq_t = pw.tile([128, CH], F32, tag="bq")
            nc.sync.dma_start(bq_t[:], bqr[:, :])
            xq0_t = load8(xq0, S, "xf")              # reuse xf slots (dead)
            cq0 = proj(xq0_t, wq_t, bq_t, "cq0")
            xq1_t = load8(xq1, S, "xl")
            cq1 = proj(xq1_t, wq_t, bq_t, "cq1")
            cq = [cq0, cq1]

            partials(ckf, cq, rs_in_f, "lf")
            nc.gpsimd.collective_compute(
                "ReduceScatter",
                mybir.AluOpType.add,
                ins=[rs_in_f[:]],
                outs=[rs_out_f[:]],
                replica_groups=groups4,
            )

            xl_t = load8(xl, S, "xl")                # reuse xq1 slots (dead)
            wk2_t = load8(wk2, CH, "wk2")
            bias2_t = pw.tile([128, CH], F32, tag="bk2")
            nc.sync.dma_start(bias2_t[:], bk2r[:, :])
            ckl = proj(xl_t, wk2_t, bias2_t, "ckl")

            partials(ckl, cq, rs_in_l, "ll")
            nc.gpsimd.collective_compute(
                "ReduceScatter",
                mybir.AluOpType.add,
                ins=[rs_in_l[:]],
                outs=[rs_out_l[:]],
                replica_groups=groups4,
            )

            # ---- V projection (local): V[J', hw_d] in bf16, bias per J' ------
            xm_t = load8(xm, S, "xm")
            wv_t = load8(wv, C, "wv")
            bv_t = pw.tile([128, 8], F32, tag="bv")
            nc.sync.dma_start(bv_t[:], bvp[:, :])
            v_sb = []
            v_first_mm = None
            for j in range(8):
                ps = pps.tile([128, S], F32, tag="mm")
                for c in range(8):
                    mm = nc.tensor.matmul(
                        ps[:],
                        wv_t[c][:, 128 * j:128 * (j + 1)],
                        xm_t[c][:],
                        start=(c == 0),
                        stop=(c == 7),
                    )
                    if v_first_mm is None:
                        v_first_mm = mm
                v = pw.tile([128, S], BF16, tag=f"v{j}")
                nc.vector.tensor_scalar_add(v[:], ps[:], bv_t[:, j:j + 1])
                v_sb.append(v)

            # ---- residual: R[e] = 0.5 * (xf + xl) on permuted rows -----------
            r_sb = []
            for e in range(8):
                a = pw.tile([128, S], F32, tag=f"wk1{e % 4}")
                nc.sync.dma_start(a[:], xfp[128 * e:128 * (e + 1), :])
                b = pw.tile([128, S], F32, tag=f"wk1{4 + e % 4}")
                nc.sync.dma_start(b[:], xlp[128 * e:128 * (e + 1), :])
                r = pw.tile([128, S], F32, tag=f"wk2{e}")
                nc.vector.tensor_add(r[:], a[:], b[:])
                nc.scalar.mul(r[:], r[:], 0.5)
                r_sb.append(r)

            # ---- softmax on the dealt 128-row block -------------------------
            att_parts = []
            for (ci, slot), rout in (((0, "wq0"), rs_out_f), ((1, "wq1"), rs_out_l)):
                lg = pw.tile([128, C], F32, tag=f"wv{ci}")
                nc.sync.dma_start(lg[:], rout[:, :])
                mxn = psc.tile([128, 1], F32, tag="mx")
                nc.vector.reduce_max(
                    mxn[:], lg[:], axis=mybir.AxisListType.X, negate=True
                )
                sm = psc.tile([128, 1], F32, tag="sm")
                nc.scalar.activation(
                    lg[:],
                    lg[:],
                    mybir.ActivationFunctionType.Exp,
                    bias=mxn[:, 0:1],
                    accum_out=sm[:, 0:1],
                )
                rcp = psc.tile([128, 1], F32, tag="rc")
                nc.vector.reciprocal(rcp[:], sm[:])
                at = pw.tile([128, C], BF16, tag=slot)
                nc.vector.tensor_scalar_mul(at[:], lg[:], rcp[:, 0:1])
                att_parts.append(at)
            att_sum = pw.tile([128, C], BF16, tag="wq2")
            nc.vector.tensor_add(att_sum[:], att_parts[0][:], att_parts[1][:])
            att_dma = nc.sync.dma_start(att_in[:], att_sum[:])
            add_dep_helper(att_dma.ins, v_first_mm.ins, sync=True,
                           reason="run V proj inside the att AllGather window")
            nc.gpsimd.collective_compute(
                "AllGather",
                mybir.AluOpType.bypass,
                ins=[att_in[:]],
                outs=[att_out[:]],
                replica_groups=groups8,
            )

            # ---- out[:, hw_d] = att @ V_d + R -------------------------------
            att_t = []
            for k in range(8):
                t = pw.tile([128, C], BF16, tag=f"xm{k}")
                nc.sync.dma_start(
                    t[:], att_out[:, 128 * k:128 * (k + 1)], transpose=True
                )
                att_t.append(t)
            out_v = out_ext[:].rearrange("(o t) w -> t o w", t=2)
            for e in range(8):
                ps = pps.tile([128, S], F32, tag="mm")
                for k in range(8):
                    nc.tensor.matmul(
                        ps[:],
                        att_t[k][:, 128 * e:128 * (e + 1)],
                        v_sb[k][:],
                        start=(k == 0),
                        stop=(k == 7),
                    )
                ost = pw.tile([128, S], F32, tag=f"wq{3 + e % 2}")
                nc.vector.tensor_add(ost[:], ps[:], r_sb[e][:])
                nc.sync.dma_start(
                    out_v[e // 4, 128 * (e % 4):128 * (e % 4 + 1), :], ost[:]
                )

    nc.compile()
    return nc


def _prep_inputs(x_f, x_m, x_l, Wq, bq, Wk1, bk1, Wk2, bk2, Wv, bv, gamma):
    Xf = np.ascontiguousarray(x_f.reshape(C, HW), dtype=np.float32)
    Xm = np.ascontiguousarray(x_m.reshape(C, HW), dtype=np.float32)
    Xl = np.ascontiguousarray(x_l.reshape(C, HW), dtype=np.float32)
    g = np.float32(np.asarray(gamma).reshape(-1)[0])

    permJ = 2 * (np.arange(C) % 512) + np.arange(C) // 512  # J' -> global j
    wv_full = np.ascontiguousarray((g * Wv)[permJ, :].T, dtype=np.float32)
    bv_perm = (g * bv)[permJ].astype(np.float32)

    wq_full = np.ascontiguousarray(Wq.T, dtype=np.float32)
    wk1_full = np.ascontiguousarray(Wk1.T, dtype=np.float32)
    wk2_full = np.ascontiguousarray(Wk2.T, dtype=np.float32)
    bqr = np.ascontiguousarray(np.broadcast_to(bq, (128, CH)), dtype=np.float32)
    bk1r = np.ascontiguousarray(np.broadcast_to(bk1, (128, CH)), dtype=np.float32)
    bk2r = np.ascontiguousarray(np.broadcast_to(bk2, (128, CH)), dtype=np.float32)
    bvp = np.ascontiguousarray(bv_perm.reshape(8, 128).T)
    Xfp = Xf[permJ, :]
    Xlp = Xl[permJ, :]

    in_maps = []
    for d in range(NCORES):
        sl = slice(S * d, S * (d + 1))
        s0 = slice(S * (d % 4), S * (d % 4 + 1))
        s1 = slice(S * (4 + d % 4), S * (4 + d % 4 + 1))
        in_maps.append({
            "xm": np.ascontiguousarray(Xm[:, sl]),
            "xf": np.ascontiguousarray(Xf[:, sl]),
            "xl": np.ascontiguousarray(Xl[:, sl]),
            "xq0": np.ascontiguousarray(Xm[:, s0]),
            "xq1": np.ascontiguousarray(Xm[:, s1]),
            "wq": wq_full,
            "wk1": wk1_full,
            "wk2": wk2_full,
            "wv": wv_full,
            "bqr": bqr,
            "bk1r": bk1r,
            "bk2r": bk2r,
            "bvp": bvp,
            "xfp": np.ascontiguousarray(Xfp[:, sl]),
            "xlp": np.ascontiguousarray(Xlp[:, sl]),
        })
    return in_maps


def _run(inputs: dict, trace: bool = False, **kw):
    if "nc" not in _CACHE:
        _CACHE["nc"] = _build()
    nc = _CACHE["nc"]
    in_maps = _prep_inputs(**inputs)
    res = run_bass_kernel_spmd(nc, in_maps, list(range(NCORES)), trace=trace, **kw)
    out = np.empty((C, HW), np.float32)
    for d in range(NCORES):
        out[:, S * d:S * (d + 1)] = res.results[d]["out"]
    return out.reshape(1, C, 64, 64), res


def kernel(**inputs) -> np.ndarray:
    inputs = {k: np.asarray(v) for k, v in inputs.items()}
    out, _ = _run(inputs)
    return out


# revision 24
# speedup vs baseline: 1.0040x; 1.0040x over previous
"""CoAtten2 Trainium2 kernel: 8-way tensor-parallel over one TRN2 chip.

Reference computation (C=1024, H=W=64, HW=4096):
    q   = (Wq @ Xm + bq)  viewed [1024, 2048] then transposed
    kf  = (Wk1 @ Xf + bk1) viewed [1024, 2048]
    kl  = (Wk2 @ Xl + bk2) viewed [1024, 2048]
    att = softmax(kf @ q) + softmax(kl @ q)          # [1024, 1024]
    out = gamma * (att @ (Wv @ Xm + bv)) + (Xf + Xl)/2

Decomposition (per core d of 8; group t = d//4, a = d%4):
  - Channel indices are permuted (I' = 512t + o <-> i = 2o + t) so the
    torch-style reshape becomes contiguous; the permutation is folded into the
    host-side Wv/bv prep and the output DMA access pattern; gamma into Wv/bv.
  - logits_PERM splits into parity quadrants Q(t, t') whose kf operand needs
    spatial columns [2048t, 2048t+2048) and whose q operand needs spatial
    columns [2048t', ...). Core d owns spatial slice S_d = [512d, 512(d+1));
    it computes the partial contraction over S_d of Q(t=d//4, t'=0 and 1) for
    both attention branches, using ONLY local projections:
      CkfT_d / CklT_d from its own x slices, and CqT over spatial blocks
      (d%4) and (4 + d%4) from two host-provided Xm slices (xq0/xq1).
  - One 4-way ReduceScatter (groups [0-3], [4-7]) then sums the four partials
    AND deals each core its 128-row attention block at a fixed local address.
  - softmax is a free-dim reduction; the summed attention is AllGathered in
    bf16; each core computes its output column slice att @ V_d + residual.
  - Collectives: one ReduceScatter (4 MB) + one small AllGather. All compute
    is local; the SPMD program is identical on every core (per-core identity
    enters only through input data).
"""

import sys

sys.path.insert(0, "/opt/trn_rl_repo")

import numpy as np

import concourse.bacc as bacc
import concourse.mybir as mybir
from concourse import tile
from concourse.tile import add_dep_helper
from concourse.bass_utils import run_bass_kernel_spmd

F32 = mybir.dt.float32
F32R = mybir.dt.float32r
BF16 = mybir.dt.bfloat16

C = 1024
HW = 4096
S = 512          # spatial columns per core
CH = 512         # C // 2 (projection output channels)
NCORES = 8

_CACHE: dict = {}


def _build():
    nc = bacc.Bacc("TRN2", target_bir_lowering=False, debug=False, num_devices=NCORES)

    # per-core external inputs
    xm = nc.declare_dram_parameter("xm", [C, S], F32, isOutput=False)
    xf = nc.declare_dram_parameter("xf", [C, S], F32, isOutput=False)
    xl = nc.declare_dram_parameter("xl", [C, S], F32, isOutput=False)
    xq0 = nc.declare_dram_parameter("xq0", [C, S], F32, isOutput=False)  # Xm block d%4
    xq1 = nc.declare_dram_parameter("xq1", [C, S], F32, isOutput=False)  # Xm block 4+d%4
    wq = nc.declare_dram_parameter("wq", [C, CH], F32, isOutput=False)   # Wq.T
    wk1 = nc.declare_dram_parameter("wk1", [C, CH], F32, isOutput=False)
    wk2 = nc.declare_dram_parameter("wk2", [C, CH], F32, isOutput=False)
    wv = nc.declare_dram_parameter("wv", [C, C], F32, isOutput=False)    # (g*Wv)[permJ].T
    bqr = nc.declare_dram_parameter("bqr", [128, CH], F32, isOutput=False)
    bk1r = nc.declare_dram_parameter("bk1r", [128, CH], F32, isOutput=False)
    bk2r = nc.declare_dram_parameter("bk2r", [128, CH], F32, isOutput=False)
    bvp = nc.declare_dram_parameter("bvp", [128, 8], F32, isOutput=False)
    xfp = nc.declare_dram_parameter("xfp", [C, S], F32, isOutput=False)  # perm rows
    xlp = nc.declare_dram_parameter("xlp", [C, S], F32, isOutput=False)
    out_ext = nc.declare_dram_parameter("out", [C, S], F32, isOutput=True)

    # internal DRAM
    rs_in_f = nc.dram_tensor("rs_in_f", [CH, C], F32)   # quadrant partials
    rs_in_l = nc.dram_tensor("rs_in_l", [CH, C], F32)
    rs_out_f = nc.dram_tensor("rs_out_f", [128, C], F32)
    rs_out_l = nc.dram_tensor("rs_out_l", [128, C], F32)
    att_in = nc.dram_tensor("att_in", [128, C], BF16)
    att_out = nc.dram_tensor("att_out", [C, C], BF16, addr_space="Shared")

    groups8 = [list(range(NCORES))]
    groups4 = [[0, 1, 2, 3], [4, 5, 6, 7]]

    with tile.TileContext(nc) as tc:
        with (
            tc.tile_pool(name="pw", bufs=1) as pw,
            tc.tile_pool(name="psg", bufs=2) as psg,
            tc.tile_pool(name="psc", bufs=2) as psc,
            tc.tile_pool(name="pps", bufs=2, space="PSUM") as pps,
            tc.tile_pool(name="plog", bufs=1, space="PSUM") as plog,
        ):
            # ---- input loads -------------------------------------------------
            def load8(dram, width, tag, dt=F32R):
                ts = []
                for c in range(8):
                    t = pw.tile([128, width], dt, tag=f"{tag}{c}")
                    nc.sync.dma_start(t[:], dram[128 * c:128 * (c + 1), :].bitcast(dt))
                    ts.append(t)
                return ts

            # ---- local transposed projections -------------------------------
            # proj(X, WT, b)[s, o] = sum_c X[c, s] WT[c, o] + b[o]  -> [512, 512]
            # result stays in SBUF as 4 [128, 512] f32r tiles (s on partitions).
            def proj(x_tiles, w_tiles, bias_t, otag):
                outs = []
                for ssub in range(4):
                    ps = pps.tile([128, CH], F32, tag="mm")
                    for c in range(8):
                        nc.tensor.matmul(
                            ps[:],
                            x_tiles[c][:, 128 * ssub:128 * (ssub + 1)],
                            w_tiles[c][:],
                            start=(c == 0),
                            stop=(c == 7),
                        )
                    o = pw.tile([128, CH], F32R, tag=f"{otag}{ssub}")
                    nc.vector.tensor_add(o[:], ps[:], bias_t[:])
                    outs.append(o)
                return outs

            def partials(ck, cq, rin, ptag):
                # For o-tile m: partial[128 o, 512 t'-block] over local s
                for m in range(4):
                    psl = plog.tile([128, C], F32, tag=ptag)
                    for tp in range(2):
                        for k in range(4):
                            nc.tensor.matmul(
                                psl[:, CH * tp:CH * (tp + 1)],
                                ck[k][:, 128 * m:128 * (m + 1)],
                                cq[tp][k][:],
                                start=(k == 0),
                                stop=(k == 3),
                            )
                    stg = psg.tile([128, C], F32, tag="stg")
                    nc.vector.tensor_copy(stg[:], psl[:])
                    nc.sync.dma_start(rin[128 * m:128 * (m + 1), :], stg[:])

            # f-branch chain first: its partials run while the l-branch's
            # inputs load and project
            xf_t = load8(xf, S, "xf")
            wk1_t = load8(wk1, CH, "wk1")
            bias_t = pw.tile([128, CH], F32, tag="bk1")
            nc.sync.dma_start(bias_t[:], bk1r[:, :])
            ckf = proj(xf_t, wk1_t, bias_t, "ckf")

            wq_t = load8(wq, CH, "wq")
            bq_t = pw.tile([128, CH], F32, tag="bq")
            nc.sync.dma_start(bq_t[:], bqr[:, :])
            xq0_t = load8(xq0, S, "xf")              # reuse xf slots (dead)
            cq0 = proj(xq0_t, wq_t, bq_t, "cq0")
            xq1_t = load8(xq1, S, "xl")
            cq1 = proj(xq1_t, wq_t, bq_t, "cq1")
            cq = [cq0, cq1]

            partials(ckf, cq, rs_in_f, "lf")
            nc.gpsimd.collective_compute(
                "ReduceScatter",
                mybir.AluOpType.add,
                ins=[rs_in_f[:]],
                outs=[rs_out_f[:]],
                replica_groups=groups4,
            )

            xl_t = load8(xl, S, "xl")                # reuse xq1 slots (dead)
            wk2_t = load8(wk2, CH, "wk2")
            bias2_t = pw.tile([128, CH], F32, tag="bk2")
            nc.sync.dma_start(bias2_t[:], bk2r[:, :])
            ckl = proj(xl_t, wk2_t, bias2_t, "ckl")

            partials(ckl, cq, rs_in_l, "ll")
            nc.gpsimd.collective_compute(
                "ReduceScatter",
                mybir.AluOpType.add,
                ins=[rs_in_l[:]],
                outs=[rs_out_l[:]],
                replica_groups=groups4,
            )

            # ---- V projection (local): V[J', hw_d] in bf16, bias per J' ------
            xm_t = load8(xm, S, "xm")
            wv_t = load8(wv, C, "wv")
            bv_t = pw.tile([128, 8], F32, tag="bv")
            nc.sync.dma_start(bv_t[:], bvp[:, :])
            v_sb = []
            v_first_mm = None
            for j in range(8):
                ps = pps.tile([128, S], F32, tag="mm")
                for c in range(8):
                    mm = nc.tensor.matmul(
                        ps[:],
                        wv_t[c][:, 128 * j:128 * (j + 1)],
                        xm_t[c][:],
                        start=(c == 0),
                        stop=(c == 7),
                    )
                    if v_first_mm is None:
                        v_first_mm = mm
                v = pw.tile([128, S], BF16, tag=f"v{j}")
                nc.vector.tensor_scalar_add(v[:], ps[:], bv_t[:, j:j + 1])
                v_sb.append(v)

            # ---- residual: R[e] = 0.5 * (xf + xl) on permuted rows -----------
            r_sb = []
            for e in range(8):
                a = pw.tile([128, S], F32, tag=f"wk1{e % 4}")
                nc.sync.dma_start(a[:], xfp[128 * e:128 * (e + 1), :])
                b = pw.tile([128, S], F32, tag=f"wk1{4 + e % 4}")
                nc.sync.dma_start(b[:], xlp[128 * e:128 * (e + 1), :])
                r = pw.tile([128, S], F32, tag=f"wk2{e}")
                nc.vector.tensor_add(r[:], a[:], b[:])
                nc.scalar.mul(r[:], r[:], 0.5)
                r_sb.append(r)

            # ---- softmax on the dealt 128-row block -------------------------
            att_parts = []
            for (ci, slot), rout in (((0, "wq0"), rs_out_f), ((1, "wq1"), rs_out_l)):
                lg = pw.tile([128, C], F32, tag=f"wv{ci}")
                nc.sync.dma_start(lg[:], rout[:, :])
                mxn = psc.tile([128, 1], F32, tag="mx")
                nc.vector.reduce_max(
                    mxn[:], lg[:], axis=mybir.AxisListType.X, negate=True
                )
                sm = psc.tile([128, 1], F32, tag="sm")
                nc.scalar.activation(
                    lg[:],
                    lg[:],
                    mybir.ActivationFunctionType.Exp,
                    bias=mxn[:, 0:1],
                    accum_out=sm[:, 0:1],
                )
                rcp = psc.tile([128, 1], F32, tag="rc")
                nc.vector.reciprocal(rcp[:], sm[:])
                at = pw.tile([128, C], BF16, tag=slot)
                nc.vector.tensor_scalar_mul(at[:], lg[:], rcp[:, 0:1])
                att_parts.append(at)
            att_sum = pw.tile([128, C], BF16, tag="wq2")
            nc.vector.tensor_add(att_sum[:], att_parts[0][:], att_parts[1][:])
            att_dma = nc.sync.dma_start(att_in[:], att_sum[:])
            add_dep_helper(att_dma.ins, v_first_mm.ins, sync=True,
                           reason="run V proj inside the att AllGather window")
            nc.gpsimd.collective_compute(
                "AllGather",
                mybir.AluOpType.bypass,
                ins=[att_in[:]],
                outs=[att_out[:]],
                replica_groups=groups8,
            )

            # ---- out[:, hw_d] = att @ V_d + R -------------------------------
            att_t = []
            for k in range(8):
                t = pw.tile([128, C], BF16, tag=f"xm{k}")
                nc.sync.dma_start(
                    t[:], att_out[:, 128 * k:128 * (k + 1)], transpose=True
                )
                att_t.append(t)
            out_v = out_ext[:].rearrange("(o t) w -> t o w", t=2)
            for e in range(8):
                ps = pps.tile([128, S], F32, tag="mm")
                for k in range(8):
                    nc.tensor.matmul(
                        ps[:],
                        att_t[k][:, 128 * e:128 * (e + 1)],
                        v_sb[k][:],
                        start=(k == 0),
                        stop=(k == 7),
                    )
                ost = pw.tile([128, S], F32, tag=f"wq{3 + e % 2}")
                nc.vector.tensor_add(ost[:], ps[:], r_sb[e][:])
                nc.sync.dma_start(
                    out_v[e // 4, 128 * (e % 4):128 * (e % 4 + 1), :], ost[:]
                )

    nc.compile()
    return nc


def _prep_inputs(x_f, x_m, x_l, Wq, bq, Wk1, bk1, Wk2, bk2, Wv, bv, gamma):
    Xf = np.ascontiguousarray(x_f.reshape(C, HW), dtype=np.float32)
    Xm = np.ascontiguousarray(x_m.reshape(C, HW), dtype=np.float32)
    Xl = np.ascontiguousarray(x_l.reshape(C, HW), dtype=np.float32)
    g = np.float32(np.asarray(gamma).reshape(-1)[0])

    permJ = 2 * (np.arange(C) % 512) + np.arange(C) // 512  # J' -> global j
    wv_full = np.ascontiguousarray((g * Wv)[permJ, :].T, dtype=np.float32)
    bv_perm = (g * bv)[permJ].astype(np.float32)

    wq_full = np.ascontiguousarray(Wq.T, dtype=np.float32)
    wk1_full = np.ascontiguousarray(Wk1.T, dtype=np.float32)
    wk2_full = np.ascontiguousarray(Wk2.T, dtype=np.float32)
    bqr = np.ascontiguousarray(np.broadcast_to(bq, (128, CH)), dtype=np.float32)
    bk1r = np.ascontiguousarray(np.broadcast_to(bk1, (128, CH)), dtype=np.float32)
    bk2r = np.ascontiguousarray(np.broadcast_to(bk2, (128, CH)), dtype=np.float32)
    bvp = np.ascontiguousarray(bv_perm.reshape(8, 128).T)
    Xfp = Xf[permJ, :]
    Xlp = Xl[permJ, :]

    in_maps = []
    for d in range(NCORES):
        sl = slice(S * d, S * (d + 1))
        s0 = slice(S * (d % 4), S * (d % 4 + 1))
        s1 = slice(S * (4 + d % 4), S * (4 + d % 4 + 1))
        in_maps.append({
            "xm": np.ascontiguousarray(Xm[:, sl]),
            "xf": np.ascontiguousarray(Xf[:, sl]),
            "xl": np.ascontiguousarray(Xl[:, sl]),
            "xq0": np.ascontiguousarray(Xm[:, s0]),
            "xq1": np.ascontiguousarray(Xm[:, s1]),
            "wq": wq_full,
            "wk1": wk1_full,
            "wk2": wk2_full,
            "wv": wv_full,
            "bqr": bqr,
            "bk1r": bk1r,
            "bk2r": bk2r,
            "bvp": bvp,
            "xfp": np.ascontiguousarray(Xfp[:, sl]),
            "xlp": np.ascontiguousarray(Xlp[:, sl]),
        })
    return in_maps


def _run(inputs: dict, trace: bool = False, **kw):
    if "nc" not in _CACHE:
        _CACHE["nc"] = _build()
    nc = _CACHE["nc"]
    in_maps = _prep_inputs(**inputs)
    res = run_bass_kernel_spmd(nc, in_maps, list(range(NCORES)), trace=trace, **kw)
    out = np.empty((C, HW), np.float32)
    for d in range(NCORES):
        out[:, S * d:S * (d + 1)] = res.results[d]["out"]
    return out.reshape(1, C, 64, 64), res


def kernel(**inputs) -> np.ndarray:
    inputs = {k: np.asarray(v) for k, v in inputs.items()}
    out, _ = _run(inputs)
    return out


# revision 25
# speedup vs baseline: 1.0070x; 1.0030x over previous
"""CoAtten2 Trainium2 kernel: 8-way tensor-parallel over one TRN2 chip.

Reference computation (C=1024, H=W=64, HW=4096):
    q   = (Wq @ Xm + bq)  viewed [1024, 2048] then transposed
    kf  = (Wk1 @ Xf + bk1) viewed [1024, 2048]
    kl  = (Wk2 @ Xl + bk2) viewed [1024, 2048]
    att = softmax(kf @ q) + softmax(kl @ q)          # [1024, 1024]
    out = gamma * (att @ (Wv @ Xm + bv)) + (Xf + Xl)/2

Decomposition (per core d of 8; group t = d//4, a = d%4):
  - Channel indices are permuted (I' = 512t + o <-> i = 2o + t) so the
    torch-style reshape becomes contiguous; the permutation is folded into the
    host-side Wv/bv prep and the output DMA access pattern; gamma into Wv/bv.
  - logits_PERM splits into parity quadrants Q(t, t') whose kf operand needs
    spatial columns [2048t, 2048t+2048) and whose q operand needs spatial
    columns [2048t', ...). Core d owns spatial slice S_d = [512d, 512(d+1));
    it computes the partial contraction over S_d of Q(t=d//4, t'=0 and 1) for
    both attention branches, using ONLY local projections:
      CkfT_d / CklT_d from its own x slices, and CqT over spatial blocks
      (d%4) and (4 + d%4) from two host-provided Xm slices (xq0/xq1).
  - One 4-way ReduceScatter (groups [0-3], [4-7]) then sums the four partials
    AND deals each core its 128-row attention block at a fixed local address.
  - softmax is a free-dim reduction; the summed attention is AllGathered in
    bf16; each core computes its output column slice att @ V_d + residual.
  - Collectives: one ReduceScatter (4 MB) + one small AllGather. All compute
    is local; the SPMD program is identical on every core (per-core identity
    enters only through input data).
"""

import sys

sys.path.insert(0, "/opt/trn_rl_repo")

import numpy as np

import concourse.bacc as bacc
import concourse.mybir as mybir
from concourse import tile
from concourse.tile import add_dep_helper
from concourse.bass_utils import run_bass_kernel_spmd

F32 = mybir.dt.float32
F32R = mybir.dt.float32r
BF16 = mybir.dt.bfloat16

C = 1024
HW = 4096
S = 512          # spatial columns per core
CH = 512         # C // 2 (projection output channels)
NCORES = 8

_CACHE: dict = {}


def _build():
    nc = bacc.Bacc("TRN2", target_bir_lowering=False, debug=False, num_devices=NCORES)

    # per-core external inputs
    xm = nc.declare_dram_parameter("xm", [C, S], F32, isOutput=False)
    xf = nc.declare_dram_parameter("xf", [C, S], F32, isOutput=False)
    xl = nc.declare_dram_parameter("xl", [C, S], F32, isOutput=False)
    xq0 = nc.declare_dram_parameter("xq0", [C, S], F32, isOutput=False)  # Xm block d%4
    xq1 = nc.declare_dram_parameter("xq1", [C, S], F32, isOutput=False)  # Xm block 4+d%4
    wq = nc.declare_dram_parameter("wq", [C, CH], F32, isOutput=False)   # Wq.T
    wk1 = nc.declare_dram_parameter("wk1", [C, CH], F32, isOutput=False)
    wk2 = nc.declare_dram_parameter("wk2", [C, CH], F32, isOutput=False)
    wv = nc.declare_dram_parameter("wv", [C, C], F32, isOutput=False)    # (g*Wv)[permJ].T
    bqr = nc.declare_dram_parameter("bqr", [128, CH], F32, isOutput=False)
    bk1r = nc.declare_dram_parameter("bk1r", [128, CH], F32, isOutput=False)
    bk2r = nc.declare_dram_parameter("bk2r", [128, CH], F32, isOutput=False)
    bvp = nc.declare_dram_parameter("bvp", [128, 8], F32, isOutput=False)
    xfp = nc.declare_dram_parameter("xfp", [C, S], F32, isOutput=False)  # perm rows
    xlp = nc.declare_dram_parameter("xlp", [C, S], F32, isOutput=False)
    out_ext = nc.declare_dram_parameter("out", [C, S], F32, isOutput=True)

    # internal DRAM
    rs_in_f = nc.dram_tensor("rs_in_f", [CH, C], F32)   # quadrant partials
    rs_in_l = nc.dram_tensor("rs_in_l", [CH, C], F32)
    rs_out_f = nc.dram_tensor("rs_out_f", [128, C], F32)
    rs_out_l = nc.dram_tensor("rs_out_l", [128, C], F32)
    att_in = nc.dram_tensor("att_in", [128, C], BF16)
    att_out = nc.dram_tensor("att_out", [C, C], BF16, addr_space="Shared")

    groups8 = [list(range(NCORES))]
    groups4 = [[0, 1, 2, 3], [4, 5, 6, 7]]

    with tile.TileContext(nc) as tc:
        with (
            tc.tile_pool(name="pw", bufs=1) as pw,
            tc.tile_pool(name="psg", bufs=2) as psg,
            tc.tile_pool(name="psc", bufs=2) as psc,
            tc.tile_pool(name="pps", bufs=2, space="PSUM") as pps,
            tc.tile_pool(name="plog", bufs=1, space="PSUM") as plog,
        ):
            # ---- input loads -------------------------------------------------
            def load8(dram, width, tag, dt=F32R):
                ts = []
                for c in range(8):
                    t = pw.tile([128, width], dt, tag=f"{tag}{c}")
                    nc.sync.dma_start(t[:], dram[128 * c:128 * (c + 1), :].bitcast(dt))
                    ts.append(t)
                return ts

            # ---- local transposed projections -------------------------------
            # proj(X, WT, b)[s, o] = sum_c X[c, s] WT[c, o] + b[o]  -> [512, 512]
            # result stays in SBUF as 4 [128, 512] f32r tiles (s on partitions).
            def proj(x_tiles, w_tiles, bias_t, otag):
                outs = []
                for ssub in range(4):
                    ps = pps.tile([128, CH], F32, tag="mm")
                    for c in range(8):
                        nc.tensor.matmul(
                            ps[:],
                            x_tiles[c][:, 128 * ssub:128 * (ssub + 1)],
                            w_tiles[c][:],
                            start=(c == 0),
                            stop=(c == 7),
                        )
                    o = pw.tile([128, CH], F32R, tag=f"{otag}{ssub}")
                    nc.vector.tensor_add(o[:], ps[:], bias_t[:])
                    outs.append(o)
                return outs

            def partials(ck, cq, rin, ptag):
                # For o-tile m: partial[128 o, 512 t'-block] over local s
                for m in range(4):
                    psl = plog.tile([128, C], F32, tag=ptag)
                    for tp in range(2):
                        for k in range(4):
                            nc.tensor.matmul(
                                psl[:, CH * tp:CH * (tp + 1)],
                                ck[k][:, 128 * m:128 * (m + 1)],
                                cq[tp][k][:],
                                start=(k == 0),
                                stop=(k == 3),
                            )
                    stg = psg.tile([128, C], F32, tag="stg")
                    nc.vector.tensor_copy(stg[:], psl[:])
                    nc.sync.dma_start(rin[128 * m:128 * (m + 1), :], stg[:])

            # f-branch chain first: its partials run while the l-branch's
            # inputs load and project
            xf_t = load8(xf, S, "xf")
            wk1_t = load8(wk1, CH, "wk1")
            bias_t = pw.tile([128, CH], F32, tag="bk1")
            nc.sync.dma_start(bias_t[:], bk1r[:, :])
            ckf = proj(xf_t, wk1_t, bias_t, "ckf")

            wq_t = load8(wq, CH, "wq")
            bq_t = pw.tile([128, CH], F32, tag="bq")
            nc.sync.dma_start(bq_t[:], bqr[:, :])
            xq0_t = load8(xq0, S, "xf")              # reuse xf slots (dead)
            cq0 = proj(xq0_t, wq_t, bq_t, "cq0")
            xq1_t = load8(xq1, S, "xl")
            cq1 = proj(xq1_t, wq_t, bq_t, "cq1")
            cq = [cq0, cq1]

            partials(ckf, cq, rs_in_f, "lf")
            nc.gpsimd.collective_compute(
                "ReduceScatter",
                mybir.AluOpType.add,
                ins=[rs_in_f[:]],
                outs=[rs_out_f[:]],
                replica_groups=groups4,
            )

            xl_t = load8(xl, S, "xl")                # reuse xq1 slots (dead)
            wk2_t = load8(wk2, CH, "wk2")
            bias2_t = pw.tile([128, CH], F32, tag="bk2")
            nc.sync.dma_start(bias2_t[:], bk2r[:, :])
            ckl = proj(xl_t, wk2_t, bias2_t, "ckl")

            partials(ckl, cq, rs_in_l, "ll")
            nc.gpsimd.collective_compute(
                "ReduceScatter",
                mybir.AluOpType.add,
                ins=[rs_in_l[:]],
                outs=[rs_out_l[:]],
                replica_groups=groups4,
            )

            # ---- V projection (local): V[J', hw_d] in bf16, bias per J' ------
            xm_t = load8(xm, S, "xm")
            wv_t = load8(wv, C, "wv")
            bv_t = pw.tile([128, 8], F32, tag="bv")
            nc.sync.dma_start(bv_t[:], bvp[:, :])
            v_sb = []
            v_first_mm = None
            for j in range(8):
                ps = pps.tile([128, S], F32, tag="mm")
                for c in range(8):
                    mm = nc.tensor.matmul(
                        ps[:],
                        wv_t[c][:, 128 * j:128 * (j + 1)],
                        xm_t[c][:],
                        start=(c == 0),
                        stop=(c == 7),
                    )
                    if v_first_mm is None:
                        v_first_mm = mm
                v = pw.tile([128, S], BF16, tag=f"v{j}")
                nc.vector.tensor_scalar_add(v[:], ps[:], bv_t[:, j:j + 1])
                v_sb.append(v)

            # ---- residual: R[e] = 0.5 * (xf + xl) on permuted rows -----------
            r_sb = []
            for e in range(8):
                a = pw.tile([128, S], F32, tag=f"wk1{e % 4}")
                nc.sync.dma_start(a[:], xfp[128 * e:128 * (e + 1), :])
                b = pw.tile([128, S], F32, tag=f"wk1{4 + e % 4}")
                nc.sync.dma_start(b[:], xlp[128 * e:128 * (e + 1), :])
                r = pw.tile([128, S], F32, tag=f"wk2{e}")
                nc.vector.tensor_add(r[:], a[:], b[:])
                nc.scalar.mul(r[:], r[:], 0.5)
                r_sb.append(r)

            # ---- softmax on the dealt 128-row block -------------------------
            att_parts = []
            for (ci, slot), rout in (((0, "wq0"), rs_out_f), ((1, "wq1"), rs_out_l)):
                lg = pw.tile([128, C], F32, tag=f"wv{ci}")
                nc.sync.dma_start(lg[:, 0:CH], rout[:, 0:CH])
                nc.sync.dma_start(lg[:, CH:C], rout[:, CH:C])
                mxn = psc.tile([128, 1], F32, tag="mx")
                nc.vector.reduce_max(
                    mxn[:], lg[:], axis=mybir.AxisListType.X, negate=True
                )
                sm = psc.tile([128, 1], F32, tag="sm")
                nc.scalar.activation(
                    lg[:],
                    lg[:],
                    mybir.ActivationFunctionType.Exp,
                    bias=mxn[:, 0:1],
                    accum_out=sm[:, 0:1],
                )
                rcp = psc.tile([128, 1], F32, tag="rc")
                nc.vector.reciprocal(rcp[:], sm[:])
                at = pw.tile([128, C], BF16, tag=slot)
                nc.vector.tensor_scalar_mul(at[:], lg[:], rcp[:, 0:1])
                att_parts.append(at)
            att_sum = pw.tile([128, C], BF16, tag="wq2")
            nc.vector.tensor_add(att_sum[:], att_parts[0][:], att_parts[1][:])
            nc.sync.dma_start(att_in[:, 0:CH], att_sum[:, 0:CH])
            att_dma = nc.sync.dma_start(att_in[:, CH:C], att_sum[:, CH:C])
            add_dep_helper(att_dma.ins, v_first_mm.ins, sync=True,
                           reason="run V proj inside the att AllGather window")
            nc.gpsimd.collective_compute(
                "AllGather",
                mybir.AluOpType.bypass,
                ins=[att_in[:]],
                outs=[att_out[:]],
                replica_groups=groups8,
            )

            # ---- out[:, hw_d] = att @ V_d + R -------------------------------
            att_t = []
            for k in range(8):
                t = pw.tile([128, C], BF16, tag=f"xm{k}")
                nc.sync.dma_start(
                    t[:], att_out[:, 128 * k:128 * (k + 1)], transpose=True
                )
                att_t.append(t)
            out_v = out_ext[:].rearrange("(o t) w -> t o w", t=2)
            for e in range(8):
                ps = pps.tile([128, S], F32, tag="mm")
                for k in range(8):
                    nc.tensor.matmul(
                        ps[:],
                        att_t[k][:, 128 * e:128 * (e + 1)],
                        v_sb[k][:],
                        start=(k == 0),
                        stop=(k == 7),
                    )
                ost = pw.tile([128, S], F32, tag=f"wq{3 + e % 2}")
                nc.vector.tensor_add(ost[:], ps[:], r_sb[e][:])
                nc.sync.dma_start(
                    out_v[e // 4, 128 * (e % 4):128 * (e % 4 + 1), :], ost[:]
                )

    nc.compile()
    return nc


def _prep_inputs(x_f, x_m, x_l, Wq, bq, Wk1, bk1, Wk2, bk2, Wv, bv, gamma):
    Xf = np.ascontiguousarray(x_f.reshape(C, HW), dtype=np.float32)
    Xm = np.ascontiguousarray(x_m.reshape(C, HW), dtype=np.float32)
    Xl = np.ascontiguousarray(x_l.reshape(C, HW), dtype=np.float32)
    g = np.float32(np.asarray(gamma).reshape(-1)[0])

    permJ = 2 * (np.arange(C) % 512) + np.arange(C) // 512  # J' -> global j
    wv_full = np.ascontiguousarray((g * Wv)[permJ, :].T, dtype=np.float32)
    bv_perm = (g * bv)[permJ].astype(np.float32)

    wq_full = np.ascontiguousarray(Wq.T, dtype=np.float32)
    wk1_full = np.ascontiguousarray(Wk1.T, dtype=np.float32)
    wk2_full = np.ascontiguousarray(Wk2.T, dtype=np.float32)
    bqr = np.ascontiguousarray(np.broadcast_to(bq, (128, CH)), dtype=np.float32)
    bk1r = np.ascontiguousarray(np.broadcast_to(bk1, (128, CH)), dtype=np.float32)
    bk2r = np.ascontiguousarray(np.broadcast_to(bk2, (128, CH)), dtype=np.float32)
    bvp = np.ascontiguousarray(bv_perm.reshape(8, 128).T)
    Xfp = Xf[permJ, :]
    Xlp = Xl[permJ, :]

    in_maps = []
    for d in range(NCORES):
        sl = slice(S * d, S * (d + 1))
        s0 = slice(S * (d % 4), S * (d % 4 + 1))
        s1 = slice(S * (4 + d % 4), S * (4 + d % 4 + 1))
        in_maps.append({
            "xm": np.ascontiguousarray(Xm[:, sl]),
            "xf": np.ascontiguousarray(Xf[:, sl]),
            "xl": np.ascontiguousarray(Xl[:, sl]),
            "xq0": np.ascontiguousarray(Xm[:, s0]),
            "xq1": np.ascontiguousarray(Xm[:, s1]),
            "wq": wq_full,
            "wk1": wk1_full,
            "wk2": wk2_full,
            "wv": wv_full,
            "bqr": bqr,
            "bk1r": bk1r,
            "bk2r": bk2r,
            "bvp": bvp,
            "xfp": np.ascontiguousarray(Xfp[:, sl]),
            "xlp": np.ascontiguousarray(Xlp[:, sl]),
        })
    return in_maps


def _run(inputs: dict, trace: bool = False, **kw):
    if "nc" not in _CACHE:
        _CACHE["nc"] = _build()
    nc = _CACHE["nc"]
    in_maps = _prep_inputs(**inputs)
    res = run_bass_kernel_spmd(nc, in_maps, list(range(NCORES)), trace=trace, **kw)
    out = np.empty((C, HW), np.float32)
    for d in range(NCORES):
        out[:, S * d:S * (d + 1)] = res.results[d]["out"]
    return out.reshape(1, C, 64, 64), res


def kernel(**inputs) -> np.ndarray:
    inputs = {k: np.asarray(v) for k, v in inputs.items()}
    out, _ = _run(inputs)
    return out


# revision 26
# speedup vs baseline: 1.0198x; 1.0127x over previous
"""CoAtten2 Trainium2 kernel: 8-way tensor-parallel over one TRN2 chip.

Reference computation (C=1024, H=W=64, HW=4096):
    q   = (Wq @ Xm + bq)  viewed [1024, 2048] then transposed
    kf  = (Wk1 @ Xf + bk1) viewed [1024, 2048]
    kl  = (Wk2 @ Xl + bk2) viewed [1024, 2048]
    att = softmax(kf @ q) + softmax(kl @ q)          # [1024, 1024]
    out = gamma * (att @ (Wv @ Xm + bv)) + (Xf + Xl)/2

Decomposition (per core d of 8; group t = d//4, a = d%4):
  - Channel indices are permuted (I' = 512t + o <-> i = 2o + t) so the
    torch-style reshape becomes contiguous; the permutation is folded into the
    host-side Wv/bv prep and the output DMA access pattern; gamma into Wv/bv.
  - logits_PERM splits into parity quadrants Q(t, t') whose kf operand needs
    spatial columns [2048t, 2048t+2048) and whose q operand needs spatial
    columns [2048t', ...). Core d owns spatial slice S_d = [512d, 512(d+1));
    it computes the partial contraction over S_d of Q(t=d//4, t'=0 and 1) for
    both attention branches, using ONLY local projections:
      CkfT_d / CklT_d from its own x slices, and CqT over spatial blocks
      (d%4) and (4 + d%4) from two host-provided Xm slices (xq0/xq1).
  - One 4-way ReduceScatter (groups [0-3], [4-7]) then sums the four partials
    AND deals each core its 128-row attention block at a fixed local address.
  - softmax is a free-dim reduction; the summed attention is AllGathered in
    bf16; each core computes its output column slice att @ V_d + residual.
  - Collectives: one ReduceScatter (4 MB) + one small AllGather. All compute
    is local; the SPMD program is identical on every core (per-core identity
    enters only through input data).
"""

import sys

sys.path.insert(0, "/opt/trn_rl_repo")

import numpy as np

import concourse.bacc as bacc
import concourse.mybir as mybir
from concourse import tile
from concourse.tile import add_dep_helper
from concourse.bass_utils import run_bass_kernel_spmd

F32 = mybir.dt.float32
F32R = mybir.dt.float32r
BF16 = mybir.dt.bfloat16

C = 1024
HW = 4096
S = 512          # spatial columns per core
CH = 512         # C // 2 (projection output channels)
NCORES = 8

_CACHE: dict = {}


def _build():
    nc = bacc.Bacc("TRN2", target_bir_lowering=False, debug=False, num_devices=NCORES)

    # per-core external inputs
    xm = nc.declare_dram_parameter("xm", [C, S], F32, isOutput=False)
    xf = nc.declare_dram_parameter("xf", [C, S], F32, isOutput=False)
    xl = nc.declare_dram_parameter("xl", [C, S], F32, isOutput=False)
    xq0 = nc.declare_dram_parameter("xq0", [C, S], F32, isOutput=False)  # Xm block d%4
    xq1 = nc.declare_dram_parameter("xq1", [C, S], F32, isOutput=False)  # Xm block 4+d%4
    wq = nc.declare_dram_parameter("wq", [C, CH], F32, isOutput=False)   # Wq.T
    wk1 = nc.declare_dram_parameter("wk1", [C, CH], F32, isOutput=False)
    wk2 = nc.declare_dram_parameter("wk2", [C, CH], F32, isOutput=False)
    wv = nc.declare_dram_parameter("wv", [C, C], F32, isOutput=False)    # (g*Wv)[permJ].T
    bqr = nc.declare_dram_parameter("bqr", [128, CH], F32, isOutput=False)
    bk1r = nc.declare_dram_parameter("bk1r", [128, CH], F32, isOutput=False)
    bk2r = nc.declare_dram_parameter("bk2r", [128, CH], F32, isOutput=False)
    bvp = nc.declare_dram_parameter("bvp", [128, 8], F32, isOutput=False)
    xfp = nc.declare_dram_parameter("xfp", [C, S], F32, isOutput=False)  # perm rows
    xlp = nc.declare_dram_parameter("xlp", [C, S], F32, isOutput=False)
    out_ext = nc.declare_dram_parameter("out", [C, S], F32, isOutput=True)

    # internal DRAM
    rs_in_f = nc.dram_tensor("rs_in_f", [CH, C], F32)   # quadrant partials
    rs_in_l = nc.dram_tensor("rs_in_l", [CH, C], F32)
    rs_out_f = nc.dram_tensor("rs_out_f", [128, C], F32)
    rs_out_l = nc.dram_tensor("rs_out_l", [128, C], F32)
    att_in = nc.dram_tensor("att_in", [128, C], BF16)
    att_out = nc.dram_tensor("att_out", [C, C], BF16, addr_space="Shared")

    groups8 = [list(range(NCORES))]
    groups4 = [[0, 1, 2, 3], [4, 5, 6, 7]]

    with tile.TileContext(nc) as tc:
        with (
            tc.tile_pool(name="pw", bufs=1) as pw,
            tc.tile_pool(name="psg", bufs=2) as psg,
            tc.tile_pool(name="psc", bufs=2) as psc,
            tc.tile_pool(name="pps", bufs=3, space="PSUM") as pps,
            tc.tile_pool(name="plog", bufs=1, space="PSUM") as plog,
        ):
            # ---- input loads -------------------------------------------------
            def load8(dram, width, tag, dt=F32R):
                ts = []
                for c in range(8):
                    t = pw.tile([128, width], dt, tag=f"{tag}{c}")
                    nc.sync.dma_start(t[:], dram[128 * c:128 * (c + 1), :].bitcast(dt))
                    ts.append(t)
                return ts

            # ---- local transposed projections -------------------------------
            # proj(X, WT, b)[s, o] = sum_c X[c, s] WT[c, o] + b[o]  -> [512, 512]
            # result stays in SBUF as 4 [128, 512] f32r tiles (s on partitions).
            def proj(x_tiles, w_tiles, bias_t, otag):
                outs = []
                for ssub in range(4):
                    ps = pps.tile([128, CH], F32, tag="mm")
                    for c in range(8):
                        nc.tensor.matmul(
                            ps[:],
                            x_tiles[c][:, 128 * ssub:128 * (ssub + 1)],
                            w_tiles[c][:],
                            start=(c == 0),
                            stop=(c == 7),
                        )
                    o = pw.tile([128, CH], F32R, tag=f"{otag}{ssub}")
                    nc.vector.tensor_add(o[:], ps[:], bias_t[:])
                    outs.append(o)
                return outs

            def partials(ck, cq, rin, ptag):
                # For o-tile m: partial[128 o, 512 t'-block] over local s
                for m in range(4):
                    psl = plog.tile([128, C], F32, tag=ptag)
                    for tp in range(2):
                        for k in range(4):
                            nc.tensor.matmul(
                                psl[:, CH * tp:CH * (tp + 1)],
                                ck[k][:, 128 * m:128 * (m + 1)],
                                cq[tp][k][:],
                                start=(k == 0),
                                stop=(k == 3),
                            )
                    stg = psg.tile([128, C], F32, tag="stg")
                    nc.vector.tensor_copy(stg[:], psl[:])
                    nc.sync.dma_start(rin[128 * m:128 * (m + 1), :], stg[:])

            # f-branch chain first: its partials run while the l-branch's
            # inputs load and project
            xf_t = load8(xf, S, "xf")
            wk1_t = load8(wk1, CH, "wk1")
            bias_t = pw.tile([128, CH], F32, tag="bk1")
            nc.sync.dma_start(bias_t[:], bk1r[:, :])
            ckf = proj(xf_t, wk1_t, bias_t, "ckf")

            wq_t = load8(wq, CH, "wq")
            bq_t = pw.tile([128, CH], F32, tag="bq")
            nc.sync.dma_start(bq_t[:], bqr[:, :])
            xq0_t = load8(xq0, S, "xf")              # reuse xf slots (dead)
            cq0 = proj(xq0_t, wq_t, bq_t, "cq0")
            xq1_t = load8(xq1, S, "xl")
            cq1 = proj(xq1_t, wq_t, bq_t, "cq1")
            cq = [cq0, cq1]

            partials(ckf, cq, rs_in_f, "lf")
            nc.gpsimd.collective_compute(
                "ReduceScatter",
                mybir.AluOpType.add,
                ins=[rs_in_f[:]],
                outs=[rs_out_f[:]],
                replica_groups=groups4,
            )

            xl_t = load8(xl, S, "xl")                # reuse xq1 slots (dead)
            wk2_t = load8(wk2, CH, "wk2")
            bias2_t = pw.tile([128, CH], F32, tag="bk2")
            nc.sync.dma_start(bias2_t[:], bk2r[:, :])
            ckl = proj(xl_t, wk2_t, bias2_t, "ckl")

            partials(ckl, cq, rs_in_l, "ll")
            nc.gpsimd.collective_compute(
                "ReduceScatter",
                mybir.AluOpType.add,
                ins=[rs_in_l[:]],
                outs=[rs_out_l[:]],
                replica_groups=groups4,
            )

            # ---- V projection (local): V[J', hw_d] in bf16, bias per J' ------
            xm_t = load8(xm, S, "xm")
            wv_t = load8(wv, C, "wv")
            bv_t = pw.tile([128, 8], F32, tag="bv")
            nc.sync.dma_start(bv_t[:], bvp[:, :])
            v_sb = []
            v_first_mm = None
            for j in range(8):
                ps = pps.tile([128, S], F32, tag="mm")
                for c in range(8):
                    mm = nc.tensor.matmul(
                        ps[:],
                        wv_t[c][:, 128 * j:128 * (j + 1)],
                        xm_t[c][:],
                        start=(c == 0),
                        stop=(c == 7),
                    )
                    if v_first_mm is None:
                        v_first_mm = mm
                v = pw.tile([128, S], BF16, tag=f"v{j}")
                nc.vector.tensor_scalar_add(v[:], ps[:], bv_t[:, j:j + 1])
                v_sb.append(v)

            # ---- residual: R[e] = 0.5 * (xf + xl) on permuted rows -----------
            r_sb = []
            for e in range(8):
                a = pw.tile([128, S], F32, tag=f"wk1{e % 4}")
                nc.sync.dma_start(a[:], xfp[128 * e:128 * (e + 1), :])
                b = pw.tile([128, S], F32, tag=f"wk1{4 + e % 4}")
                nc.sync.dma_start(b[:], xlp[128 * e:128 * (e + 1), :])
                r = pw.tile([128, S], F32, tag=f"wk2{e}")
                nc.vector.tensor_add(r[:], a[:], b[:])
                nc.scalar.mul(r[:], r[:], 0.5)
                r_sb.append(r)

            # ---- softmax on the dealt 128-row block -------------------------
            att_parts = []
            for (ci, slot), rout in (((0, "wq0"), rs_out_f), ((1, "wq1"), rs_out_l)):
                lg = pw.tile([128, C], F32, tag=f"wv{ci}")
                nc.sync.dma_start(lg[:, 0:CH], rout[:, 0:CH])
                nc.sync.dma_start(lg[:, CH:C], rout[:, CH:C])
                mxn = psc.tile([128, 1], F32, tag="mx")
                nc.vector.reduce_max(
                    mxn[:], lg[:], axis=mybir.AxisListType.X, negate=True
                )
                sm = psc.tile([128, 1], F32, tag="sm")
                nc.scalar.activation(
                    lg[:],
                    lg[:],
                    mybir.ActivationFunctionType.Exp,
                    bias=mxn[:, 0:1],
                    accum_out=sm[:, 0:1],
                )
                rcp = psc.tile([128, 1], F32, tag="rc")
                nc.vector.reciprocal(rcp[:], sm[:])
                at = pw.tile([128, C], BF16, tag=slot)
                nc.vector.tensor_scalar_mul(at[:], lg[:], rcp[:, 0:1])
                att_parts.append(at)
            att_sum = pw.tile([128, C], BF16, tag="wq2")
            nc.vector.tensor_add(att_sum[:], att_parts[0][:], att_parts[1][:])
            nc.sync.dma_start(att_in[:, 0:CH], att_sum[:, 0:CH])
            att_dma = nc.sync.dma_start(att_in[:, CH:C], att_sum[:, CH:C])
            add_dep_helper(att_dma.ins, v_first_mm.ins, sync=True,
                           reason="run V proj inside the att AllGather window")
            nc.gpsimd.collective_compute(
                "AllGather",
                mybir.AluOpType.bypass,
                ins=[att_in[:]],
                outs=[att_out[:]],
                replica_groups=groups8,
            )

            # ---- out[:, hw_d] = att @ V_d + R -------------------------------
            att_t = []
            for k in range(8):
                t = pw.tile([128, C], BF16, tag=f"xm{k}")
                nc.sync.dma_start(
                    t[:], att_out[:, 128 * k:128 * (k + 1)], transpose=True
                )
                att_t.append(t)
            out_v = out_ext[:].rearrange("(o t) w -> t o w", t=2)
            for e in range(8):
                ps = pps.tile([128, S], F32, tag="mm")
                for k in range(8):
                    nc.tensor.matmul(
                        ps[:],
                        att_t[k][:, 128 * e:128 * (e + 1)],
                        v_sb[k][:],
                        start=(k == 0),
                        stop=(k == 7),
                    )
                ost = pw.tile([128, S], F32, tag=f"wq{3 + e % 2}")
                nc.vector.tensor_add(ost[:], ps[:], r_sb[e][:])
                nc.sync.dma_start(
                    out_v[e // 4, 128 * (e % 4):128 * (e % 4 + 1), :], ost[:]
                )

    nc.compile()
    return nc


def _prep_inputs(x_f, x_m, x_l, Wq, bq, Wk1, bk1, Wk2, bk2, Wv, bv, gamma):
    Xf = np.ascontiguousarray(x_f.reshape(C, HW), dtype=np.float32)
    Xm = np.ascontiguousarray(x_m.reshape(C, HW), dtype=np.float32)
    Xl = np.ascontiguousarray(x_l.reshape(C, HW), dtype=np.float32)
    g = np.float32(np.asarray(gamma).reshape(-1)[0])

    permJ = 2 * (np.arange(C) % 512) + np.arange(C) // 512  # J' -> global j
    wv_full = np.ascontiguousarray((g * Wv)[permJ, :].T, dtype=np.float32)
    bv_perm = (g * bv)[permJ].astype(np.float32)

    wq_full = np.ascontiguousarray(Wq.T, dtype=np.float32)
    wk1_full = np.ascontiguousarray(Wk1.T, dtype=np.float32)
    wk2_full = np.ascontiguousarray(Wk2.T, dtype=np.float32)
    bqr = np.ascontiguousarray(np.broadcast_to(bq, (128, CH)), dtype=np.float32)
    bk1r = np.ascontiguousarray(np.broadcast_to(bk1, (128, CH)), dtype=np.float32)
    bk2r = np.ascontiguousarray(np.broadcast_to(bk2, (128, CH)), dtype=np.float32)
    bvp = np.ascontiguousarray(bv_perm.reshape(8, 128).T)
    Xfp = Xf[permJ, :]
    Xlp = Xl[permJ, :]

    in_maps = []
    for d in range(NCORES):
        sl = slice(S * d, S * (d + 1))
        s0 = slice(S * (d % 4), S * (d % 4 + 1))
        s1 = slice(S * (4 + d % 4), S * (4 + d % 4 + 1))
        in_maps.append({
            "xm": np.ascontiguousarray(Xm[:, sl]),
            "xf": np.ascontiguousarray(Xf[:, sl]),
            "xl": np.ascontiguousarray(Xl[:, sl]),
            "xq0": np.ascontiguousarray(Xm[:, s0]),
            "xq1": np.ascontiguousarray(Xm[:, s1]),
            "wq": wq_full,
            "wk1": wk1_full,
            "wk2": wk2_full,
            "wv": wv_full,
            "bqr": bqr,
            "bk1r": bk1r,
            "bk2r": bk2r,
            "bvp": bvp,
            "xfp": np.ascontiguousarray(Xfp[:, sl]),
            "xlp": np.ascontiguousarray(Xlp[:, sl]),
        })
    return in_maps


def _run(inputs: dict, trace: bool = False, **kw):
    if "nc" not in _CACHE:
        _CACHE["nc"] = _build()
    nc = _CACHE["nc"]
    in_maps = _prep_inputs(**inputs)
    res = run_bass_kernel_spmd(nc, in_maps, list(range(NCORES)), trace=trace, **kw)
    out = np.empty((C, HW), np.float32)
    for d in range(NCORES):
        out[:, S * d:S * (d + 1)] = res.results[d]["out"]
    return out.reshape(1, C, 64, 64), res


def kernel(**inputs) -> np.ndarray:
    inputs = {k: np.asarray(v) for k, v in inputs.items()}
    out, _ = _run(inputs)
    return out


# revision 27
# speedup vs baseline: 1.0269x; 1.0070x over previous
"""CoAtten2 Trainium2 kernel: 8-way tensor-parallel over one TRN2 chip.

Reference computation (C=1024, H=W=64, HW=4096):
    q   = (Wq @ Xm + bq)  viewed [1024, 2048] then transposed
    kf  = (Wk1 @ Xf + bk1) viewed [1024, 2048]
    kl  = (Wk2 @ Xl + bk2) viewed [1024, 2048]
    att = softmax(kf @ q) + softmax(kl @ q)          # [1024, 1024]
    out = gamma * (att @ (Wv @ Xm + bv)) + (Xf + Xl)/2

Decomposition (per core d of 8; group t = d//4, a = d%4):
  - Channel indices are permuted (I' = 512t + o <-> i = 2o + t) so the
    torch-style reshape becomes contiguous; the permutation is folded into the
    host-side Wv/bv prep and the output DMA access pattern; gamma into Wv/bv.
  - logits_PERM splits into parity quadrants Q(t, t') whose kf operand needs
    spatial columns [2048t, 2048t+2048) and whose q operand needs spatial
    columns [2048t', ...). Core d owns spatial slice S_d = [512d, 512(d+1));
    it computes the partial contraction over S_d of Q(t=d//4, t'=0 and 1) for
    both attention branches, using ONLY local projections:
      CkfT_d / CklT_d from its own x slices, and CqT over spatial blocks
      (d%4) and (4 + d%4) from two host-provided Xm slices (xq0/xq1).
  - One 4-way ReduceScatter (groups [0-3], [4-7]) then sums the four partials
    AND deals each core its 128-row attention block at a fixed local address.
  - softmax is a free-dim reduction; the summed attention is AllGathered in
    bf16; each core computes its output column slice att @ V_d + residual.
  - Collectives: one ReduceScatter (4 MB) + one small AllGather. All compute
    is local; the SPMD program is identical on every core (per-core identity
    enters only through input data).
"""

import sys

sys.path.insert(0, "/opt/trn_rl_repo")

import numpy as np

import concourse.bacc as bacc
import concourse.mybir as mybir
from concourse import tile
from concourse.tile import add_dep_helper
from concourse.bass_utils import run_bass_kernel_spmd

F32 = mybir.dt.float32
F32R = mybir.dt.float32r
BF16 = mybir.dt.bfloat16

C = 1024
HW = 4096
S = 512          # spatial columns per core
CH = 512         # C // 2 (projection output channels)
NCORES = 8

_CACHE: dict = {}


def _build():
    nc = bacc.Bacc("TRN2", target_bir_lowering=False, debug=False, num_devices=NCORES)

    # per-core external inputs
    xm = nc.declare_dram_parameter("xm", [C, S], F32, isOutput=False)
    xf = nc.declare_dram_parameter("xf", [C, S], F32, isOutput=False)
    xl = nc.declare_dram_parameter("xl", [C, S], F32, isOutput=False)
    xq0 = nc.declare_dram_parameter("xq0", [C, S], F32, isOutput=False)  # Xm block d%4
    xq1 = nc.declare_dram_parameter("xq1", [C, S], F32, isOutput=False)  # Xm block 4+d%4
    wq = nc.declare_dram_parameter("wq", [C, CH], F32, isOutput=False)   # Wq.T
    wk1 = nc.declare_dram_parameter("wk1", [C, CH], F32, isOutput=False)
    wk2 = nc.declare_dram_parameter("wk2", [C, CH], F32, isOutput=False)
    wv = nc.declare_dram_parameter("wv", [C, C], F32, isOutput=False)    # (g*Wv)[permJ].T
    bqr = nc.declare_dram_parameter("bqr", [128, CH], F32, isOutput=False)
    bk1r = nc.declare_dram_parameter("bk1r", [128, CH], F32, isOutput=False)
    bk2r = nc.declare_dram_parameter("bk2r", [128, CH], F32, isOutput=False)
    bvp = nc.declare_dram_parameter("bvp", [128, 8], F32, isOutput=False)
    xfp = nc.declare_dram_parameter("xfp", [C, S], F32, isOutput=False)  # perm rows
    xlp = nc.declare_dram_parameter("xlp", [C, S], F32, isOutput=False)
    out_ext = nc.declare_dram_parameter("out", [C, S], F32, isOutput=True)

    # internal DRAM
    rs_in_f = nc.dram_tensor("rs_in_f", [CH, C], F32)   # quadrant partials
    rs_in_l = nc.dram_tensor("rs_in_l", [CH, C], F32)
    rs_out_f = nc.dram_tensor("rs_out_f", [128, C], F32)
    rs_out_l = nc.dram_tensor("rs_out_l", [128, C], F32)
    att_in = nc.dram_tensor("att_in", [128, C], BF16)
    att_out = nc.dram_tensor("att_out", [C, C], BF16, addr_space="Shared")

    groups8 = [list(range(NCORES))]
    groups4 = [[0, 1, 2, 3], [4, 5, 6, 7]]

    with tile.TileContext(nc) as tc:
        with (
            tc.tile_pool(name="pw", bufs=1) as pw,
            tc.tile_pool(name="psg", bufs=2) as psg,
            tc.tile_pool(name="psc", bufs=2) as psc,
            tc.tile_pool(name="pps", bufs=4, space="PSUM") as pps,
            tc.tile_pool(name="plog", bufs=1, space="PSUM") as plog,
        ):
            # ---- input loads -------------------------------------------------
            def load8(dram, width, tag, dt=F32R):
                ts = []
                for c in range(8):
                    t = pw.tile([128, width], dt, tag=f"{tag}{c}")
                    nc.sync.dma_start(t[:], dram[128 * c:128 * (c + 1), :].bitcast(dt))
                    ts.append(t)
                return ts

            # ---- local transposed projections -------------------------------
            # proj(X, WT, b)[s, o] = sum_c X[c, s] WT[c, o] + b[o]  -> [512, 512]
            # result stays in SBUF as 4 [128, 512] f32r tiles (s on partitions).
            def proj(x_tiles, w_tiles, bias_t, otag):
                outs = []
                for ssub in range(4):
                    ps = pps.tile([128, CH], F32, tag="mm")
                    for c in range(8):
                        nc.tensor.matmul(
                            ps[:],
                            x_tiles[c][:, 128 * ssub:128 * (ssub + 1)],
                            w_tiles[c][:],
                            start=(c == 0),
                            stop=(c == 7),
                        )
                    o = pw.tile([128, CH], F32R, tag=f"{otag}{ssub}")
                    nc.vector.tensor_add(o[:], ps[:], bias_t[:])
                    outs.append(o)
                return outs

            def partials(ck, cq, rin, ptag):
                # For o-tile m: partial[128 o, 512 t'-block] over local s
                for m in range(4):
                    psl = plog.tile([128, C], F32, tag=ptag)
                    for tp in range(2):
                        for k in range(4):
                            nc.tensor.matmul(
                                psl[:, CH * tp:CH * (tp + 1)],
                                ck[k][:, 128 * m:128 * (m + 1)],
                                cq[tp][k][:],
                                start=(k == 0),
                                stop=(k == 3),
                            )
                    stg = psg.tile([128, C], F32, tag="stg")
                    nc.vector.tensor_copy(stg[:], psl[:])
                    nc.sync.dma_start(rin[128 * m:128 * (m + 1), :], stg[:])

            # f-branch chain first: its partials run while the l-branch's
            # inputs load and project
            xf_t = load8(xf, S, "xf")
            wk1_t = load8(wk1, CH, "wk1")
            bias_t = pw.tile([128, CH], F32, tag="bk1")
            nc.sync.dma_start(bias_t[:], bk1r[:, :])
            ckf = proj(xf_t, wk1_t, bias_t, "ckf")

            wq_t = load8(wq, CH, "wq")
            bq_t = pw.tile([128, CH], F32, tag="bq")
            nc.sync.dma_start(bq_t[:], bqr[:, :])
            xq0_t = load8(xq0, S, "xf")              # reuse xf slots (dead)
            cq0 = proj(xq0_t, wq_t, bq_t, "cq0")
            xq1_t = load8(xq1, S, "xl")
            cq1 = proj(xq1_t, wq_t, bq_t, "cq1")
            cq = [cq0, cq1]

            partials(ckf, cq, rs_in_f, "lf")
            nc.gpsimd.collective_compute(
                "ReduceScatter",
                mybir.AluOpType.add,
                ins=[rs_in_f[:]],
                outs=[rs_out_f[:]],
                replica_groups=groups4,
            )

            xl_t = load8(xl, S, "xl")                # reuse xq1 slots (dead)
            wk2_t = load8(wk2, CH, "wk2")
            bias2_t = pw.tile([128, CH], F32, tag="bk2")
            nc.sync.dma_start(bias2_t[:], bk2r[:, :])
            ckl = proj(xl_t, wk2_t, bias2_t, "ckl")

            partials(ckl, cq, rs_in_l, "ll")
            nc.gpsimd.collective_compute(
                "ReduceScatter",
                mybir.AluOpType.add,
                ins=[rs_in_l[:]],
                outs=[rs_out_l[:]],
                replica_groups=groups4,
            )

            # ---- V projection (local): V[J', hw_d] in bf16, bias per J' ------
            xm_t = load8(xm, S, "xm")
            wv_t = load8(wv, C, "wv")
            bv_t = pw.tile([128, 8], F32, tag="bv")
            nc.sync.dma_start(bv_t[:], bvp[:, :])
            v_sb = []
            v_first_mm = None
            for j in range(8):
                ps = pps.tile([128, S], F32, tag="mm")
                for c in range(8):
                    mm = nc.tensor.matmul(
                        ps[:],
                        wv_t[c][:, 128 * j:128 * (j + 1)],
                        xm_t[c][:],
                        start=(c == 0),
                        stop=(c == 7),
                    )
                    if v_first_mm is None:
                        v_first_mm = mm
                v = pw.tile([128, S], BF16, tag=f"v{j}")
                nc.vector.tensor_scalar_add(v[:], ps[:], bv_t[:, j:j + 1])
                v_sb.append(v)

            # ---- residual: R[e] = 0.5 * (xf + xl) on permuted rows -----------
            r_sb = []
            for e in range(8):
                a = pw.tile([128, S], F32, tag=f"wk1{e % 4}")
                nc.sync.dma_start(a[:], xfp[128 * e:128 * (e + 1), :])
                b = pw.tile([128, S], F32, tag=f"wk1{4 + e % 4}")
                nc.sync.dma_start(b[:], xlp[128 * e:128 * (e + 1), :])
                r = pw.tile([128, S], F32, tag=f"wk2{e}")
                nc.vector.tensor_add(r[:], a[:], b[:])
                nc.scalar.mul(r[:], r[:], 0.5)
                r_sb.append(r)

            # ---- softmax on the dealt 128-row block -------------------------
            att_parts = []
            for (ci, slot), rout in (((0, "wq0"), rs_out_f), ((1, "wq1"), rs_out_l)):
                lg = pw.tile([128, C], F32, tag=f"wv{ci}")
                nc.sync.dma_start(lg[:, 0:CH], rout[:, 0:CH])
                nc.sync.dma_start(lg[:, CH:C], rout[:, CH:C])
                mxn = psc.tile([128, 1], F32, tag="mx")
                nc.vector.reduce_max(
                    mxn[:], lg[:], axis=mybir.AxisListType.X, negate=True
                )
                sm = psc.tile([128, 1], F32, tag="sm")
                nc.scalar.activation(
                    lg[:],
                    lg[:],
                    mybir.ActivationFunctionType.Exp,
                    bias=mxn[:, 0:1],
                    accum_out=sm[:, 0:1],
                )
                rcp = psc.tile([128, 1], F32, tag="rc")
                nc.vector.reciprocal(rcp[:], sm[:])
                at = pw.tile([128, C], BF16, tag=slot)
                nc.vector.tensor_scalar_mul(at[:], lg[:], rcp[:, 0:1])
                att_parts.append(at)
            att_sum = pw.tile([128, C], BF16, tag="wq2")
            nc.vector.tensor_add(att_sum[:], att_parts[0][:], att_parts[1][:])
            nc.sync.dma_start(att_in[:, 0:CH], att_sum[:, 0:CH])
            att_dma = nc.sync.dma_start(att_in[:, CH:C], att_sum[:, CH:C])
            add_dep_helper(att_dma.ins, v_first_mm.ins, sync=True,
                           reason="run V proj inside the att AllGather window")
            nc.gpsimd.collective_compute(
                "AllGather",
                mybir.AluOpType.bypass,
                ins=[att_in[:]],
                outs=[att_out[:]],
                replica_groups=groups8,
            )

            # ---- out[:, hw_d] = att @ V_d + R -------------------------------
            att_t = []
            for k in range(8):
                t = pw.tile([128, C], BF16, tag=f"xm{k}")
                nc.sync.dma_start(
                    t[:], att_out[:, 128 * k:128 * (k + 1)], transpose=True
                )
                att_t.append(t)
            out_v = out_ext[:].rearrange("(o t) w -> t o w", t=2)
            for e in range(8):
                ps = pps.tile([128, S], F32, tag="mm")
                for k in range(8):
                    nc.tensor.matmul(
                        ps[:],
                        att_t[k][:, 128 * e:128 * (e + 1)],
                        v_sb[k][:],
                        start=(k == 0),
                        stop=(k == 7),
                    )
                ost = pw.tile([128, S], F32, tag=f"wq{3 + e % 2}")
                nc.vector.tensor_add(ost[:], ps[:], r_sb[e][:])
                nc.sync.dma_start(
                    out_v[e // 4, 128 * (e % 4):128 * (e % 4 + 1), :], ost[:]
                )

    nc.compile()
    return nc


def _prep_inputs(x_f, x_m, x_l, Wq, bq, Wk1, bk1, Wk2, bk2, Wv, bv, gamma):
    Xf = np.ascontiguousarray(x_f.reshape(C, HW), dtype=np.float32)
    Xm = np.ascontiguousarray(x_m.reshape(C, HW), dtype=np.float32)
    Xl = np.ascontiguousarray(x_l.reshape(C, HW), dtype=np.float32)
    g = np.float32(np.asarray(gamma).reshape(-1)[0])

    permJ = 2 * (np.arange(C) % 512) + np.arange(C) // 512  # J' -> global j
    wv_full = np.ascontiguousarray((g * Wv)[permJ, :].T, dtype=np.float32)
    bv_perm = (g * bv)[permJ].astype(np.float32)

    wq_full = np.ascontiguousarray(Wq.T, dtype=np.float32)
    wk1_full = np.ascontiguousarray(Wk1.T, dtype=np.float32)
    wk2_full = np.ascontiguousarray(Wk2.T, dtype=np.float32)
    bqr = np.ascontiguousarray(np.broadcast_to(bq, (128, CH)), dtype=np.float32)
    bk1r = np.ascontiguousarray(np.broadcast_to(bk1, (128, CH)), dtype=np.float32)
    bk2r = np.ascontiguousarray(np.broadcast_to(bk2, (128, CH)), dtype=np.float32)
    bvp = np.ascontiguousarray(bv_perm.reshape(8, 128).T)
    Xfp = Xf[permJ, :]
    Xlp = Xl[permJ, :]

    in_maps = []
    for d in range(NCORES):
        sl = slice(S * d, S * (d + 1))
        s0 = slice(S * (d % 4), S * (d % 4 + 1))
        s1 = slice(S * (4 + d % 4), S * (4 + d % 4 + 1))
        in_maps.append({
            "xm": np.ascontiguousarray(Xm[:, sl]),
            "xf": np.ascontiguousarray(Xf[:, sl]),
            "xl": np.ascontiguousarray(Xl[:, sl]),
            "xq0": np.ascontiguousarray(Xm[:, s0]),
            "xq1": np.ascontiguousarray(Xm[:, s1]),
            "wq": wq_full,
            "wk1": wk1_full,
            "wk2": wk2_full,
            "wv": wv_full,
            "bqr": bqr,
            "bk1r": bk1r,
            "bk2r": bk2r,
            "bvp": bvp,
            "xfp": np.ascontiguousarray(Xfp[:, sl]),
            "xlp": np.ascontiguousarray(Xlp[:, sl]),
        })
    return in_maps


def _run(inputs: dict, trace: bool = False, **kw):
    if "nc" not in _CACHE:
        _CACHE["nc"] = _build()
    nc = _CACHE["nc"]
    in_maps = _prep_inputs(**inputs)
    res = run_bass_kernel_spmd(nc, in_maps, list(range(NCORES)), trace=trace, **kw)
    out = np.empty((C, HW), np.float32)
    for d in range(NCORES):
        out[:, S * d:S * (d + 1)] = res.results[d]["out"]
    return out.reshape(1, C, 64, 64), res


def kernel(**inputs) -> np.ndarray:
    inputs = {k: np.asarray(v) for k, v in inputs.items()}
    out, _ = _run(inputs)
    return out
